# revision 5
# baseline (speedup 1.0000x reference)
"""GCN classifier (2x GCNConv + mean-pool + 2-layer MLP) on 8 Trainium2 cores.

Key algebraic restructure vs the straightforward halo-exchange design:
conv2's output is consumed ONLY through the (linear) per-graph mean-pool, so
conv2-aggregation + pool collapse into a host-precomputed pooling matrix
  wq[s, G] = ( sum_{edges s->d, batch[d]=G} dinv[s]*dinv[d]
               + 1[batch[s]=G]*dinv[s]^2 ) / cnt[G]
giving  g[G] = (wq.T @ h1) @ W2 + b2  with h1 the conv1 output.  This removes
the AllGather of conv1 activations, the per-edge gather for conv2 (gpsimd
descriptor generation dominated the first design), and conv2's per-node dense.

Sharding: nodes partitioned contiguously, core c owns dst nodes
[c*6250, (c+1)*6250) in 49 windows of 128.  conv1 aggregation: host ships
per-edge rows x[src]*dinv[s]*dinv[d] (sym-norm pre-multiplied, fp8) plus the
matching 0/1 one-hot edge->dst matrices (fp8), both grouped into 128-edge
chunks per window and padded uniformly across cores (one SPMD program).
Scatter-add = PE matmuls oh.T @ G per chunk accumulating in PSUM.  Aggregated
windows are transposed to feature-major, densed with W1 (bf16), bias+relu'd
to h1, and immediately pooled into [64, 512] PSUM accumulators via the wq
matmul.  The pool is split in two so the first AllReduce (windows 0-39)
overlaps the remaining windows; only the small second AllReduce (windows
40-48) sits on the critical path.  The tiny W2 + MLP epilogue runs
replicated; core 0's output wins.
"""

import sys
import types

import ml_dtypes
import numpy as np

try:
    import antenv  # noqa: F401

    if "antenv.axon_hooks" not in sys.modules:
        _m = types.ModuleType("antenv.axon_hooks")
        _m._hook = None
        _m.set_axon_ntff_profile_hook = lambda h: setattr(_m, "_hook", h)
        _m.get_axon_ntff_profile_hook = lambda: _m._hook
        sys.modules["antenv.axon_hooks"] = _m
except Exception:
    pass

import concourse.bacc as bacc
import concourse.mybir as mybir
import concourse.tile as tile
from concourse import bass_utils
from concourse.masks import make_identity

F32 = mybir.dt.float32
BF16 = mybir.dt.bfloat16
F8 = mybir.dt.float8e4
AF = mybir.ActivationFunctionType
OP = mybir.AluOpType

N = 50000
E = 500000
DIN = 256
DH = 512
NG = 64
DOUT = 16

NCORES = 8
SLICE = N // NCORES  # 6250
NW = (SLICE + 127) // 128  # 49 windows of 128 dst nodes
NPAD = NW * 128  # 6272
W_SPLIT = 40  # pool windows [0, W_SPLIT) AllReduce early (hidden); rest at end
PF = 2  # DMA prefetch depth (windows)

_COMPILED: dict = {}


def _preprocess(x, edge_index, batch):
    src = np.asarray(edge_index[0], dtype=np.int64)
    dst = np.asarray(edge_index[1], dtype=np.int64)
    batch = np.asarray(batch, dtype=np.int64)

    deg = (np.bincount(dst, minlength=N) + 1).astype(np.float64)
    dinv = 1.0 / np.sqrt(deg)
    cnt = np.maximum(np.bincount(batch, minlength=NG), 1).astype(np.float64)

    loops = np.arange(N, dtype=np.int64)

    # ---- conv1 edge stream (edges + self-loops) grouped by (core, window) ----
    s1 = np.concatenate([src, loops])
    d1 = np.concatenate([dst, loops])
    n1 = (dinv[s1] * dinv[d1]).astype(np.float32)
    key1 = (d1 // SLICE) * NW + (d1 % SLICE) // 128
    order1 = np.argsort(key1, kind="stable")
    ss1, dd1, nn1 = s1[order1], d1[order1], n1[order1]
    counts1 = np.bincount(key1, minlength=NCORES * NW).reshape(NCORES, NW)
    starts1 = np.zeros(NCORES * NW + 1, dtype=np.int64)
    np.cumsum(counts1.reshape(-1), out=starts1[1:])
    K1 = np.ceil(counts1.max(axis=0) / 128).astype(np.int64)  # [NW]
    C1 = int(K1.sum())
    cs = np.zeros(NW + 1, dtype=np.int64)
    np.cumsum(K1, out=cs[1:])

    meta = tuple(int(v) for v in K1)

    # ---- pooling matrix wq[s, G] (conv2 agg + mean-pool collapsed) ----
    wflat = np.bincount(
        src * NG + batch[dst], weights=dinv[src] * dinv[dst], minlength=N * NG
    )
    wmat = wflat.reshape(N, NG)
    wmat[loops, batch] += dinv * dinv
    wmat /= cnt[None, :]
    wmat = wmat.astype(np.float32)

    xf = np.asarray(x, np.float32)
    per_core = []
    for c in range(NCORES):
        src_cols = np.zeros((C1, 128), dtype=np.int64)
        norm_cols = np.zeros((C1, 128), dtype=np.float32)
        dst_cols = np.full((C1, 128), -1, dtype=np.int64)
        for w in range(NW):
            gi = c * NW + w
            e0, e1 = starts1[gi], starts1[gi + 1]
            n_e = int(e1 - e0)
            k = int(K1[w])
            sv = np.zeros(k * 128, dtype=np.int64)
            sv[:n_e] = ss1[e0:e1]
            nv = np.zeros(k * 128, dtype=np.float32)
            nv[:n_e] = nn1[e0:e1]
            dv = np.full(k * 128, -1, dtype=np.int64)
            dv[:n_e] = dd1[e0:e1] - (c * SLICE + w * 128)
            c0 = int(cs[w])
            src_cols[c0 : c0 + k] = sv.reshape(k, 128)
            norm_cols[c0 : c0 + k] = nv.reshape(k, 128)
            dst_cols[c0 : c0 + k] = dv.reshape(k, 128)
        xg = xf[src_cols.reshape(-1)] * norm_cols.reshape(-1, 1)
        x_edges = np.ascontiguousarray(
            xg.astype(ml_dtypes.float8_e4m3)
            .reshape(C1, 128, DIN)
            .transpose(1, 0, 2)
            .reshape(128, C1 * DIN)
        )
        dl = dst_cols.reshape(-1)
        ohm = np.zeros((C1 * 128, 128), dtype=ml_dtypes.float8_e4m3)
        valid = np.nonzero(dl >= 0)[0]
        ohm[valid, dl[valid]] = 1.0
        oh_cols = np.ascontiguousarray(
            ohm.reshape(C1, 128, 128).transpose(1, 0, 2).reshape(128, C1 * 128)
        )

        wc = np.zeros((NPAD, NG), dtype=np.float32)
        wc[:SLICE] = wmat[c * SLICE : (c + 1) * SLICE]
        wq = np.ascontiguousarray(
            wc.reshape(NW, 128, NG).transpose(1, 0, 2).reshape(128, NW * NG)
        ).astype(ml_dtypes.bfloat16)

        per_core.append(dict(x_edges=x_edges, oh_cols=oh_cols, wq=wq))
    return meta, per_core


def _build_program(meta):
    K1 = np.array(meta)
    C1 = int(K1.sum())
    cs = np.zeros(NW + 1, dtype=np.int64)
    np.cumsum(K1, out=cs[1:])
    KMAX = int(K1.max())

    nc = bacc.Bacc("TRN2", target_bir_lowering=False, debug=False, num_devices=NCORES)

    def din(name, shape, dt=F32):
        return nc.dram_tensor(name, shape, dt, kind="ExternalInput").ap()

    x_edges = din("x_edges", [128, C1 * DIN], F8)
    oh_cols = din("oh_cols", [128, C1 * 128], F8)
    wq = din("wq", [128, NW * NG], BF16)
    W1bf = din("W1bf", [DIN, DH], BF16)
    # packed f32 constants: [0:512] b1 replicated, [512:514] b2 cols,
    # [514:515] bf1 col, [515:531] Wf2, [531:532] bf2 (rows 0..15)
    cst = din("cst", [128, DH + 2 + 1 + DOUT + 1])
    W2 = din("W2", [DH, DH // 2])
    Wf1 = din("Wf1", [DH // 2, DH // 4])
    out = nc.dram_tensor("out", [NG, DOUT], F32, kind="ExternalOutput").ap()

    with tile.TileContext(nc) as tc:
        with (
            tc.tile_pool(name="const", bufs=1) as cp,
            tc.tile_pool(name="work", bufs=1) as wp,
            tc.tile_pool(name="psum", bufs=1, space="PSUM") as pp,
            tc.tile_pool(name="dram", bufs=1, space="DRAM") as dp,
        ):
            g1_t: dict = {}
            oh_t: dict = {}

            def issue_dma(w):
                c0 = int(cs[w])
                nch = int(K1[w])
                G1 = wp.tile([128, KMAX, DIN], F8, tag="G1", bufs=PF + 1, name=f"g1_{w}")
                nc.sync.dma_start(
                    G1[:, :nch, :].rearrange("p c d -> p (c d)"),
                    x_edges[:, c0 * DIN : (c0 + nch) * DIN],
                )
                oh = wp.tile([128, KMAX, 128], F8, tag="oh", bufs=PF + 1, name=f"oh_{w}")
                nc.sync.dma_start(
                    oh[:, :nch, :].rearrange("p c d -> p (c d)"),
                    oh_cols[:, c0 * 128 : (c0 + nch) * 128],
                )
                g1_t[w] = G1
                oh_t[w] = oh

            for w in range(PF):
                issue_dma(w)

            def load(ap_in, shape, dt=F32, pool=cp):
                t = pool.tile(shape, dt, name=ap_in.tensor.name + "_sb")
                nc.sync.dma_start(t[:], ap_in[:])
                return t

            W1b = [cp.tile([128, DH], BF16, name=f"w1b_{k}") for k in range(2)]
            for k in range(2):
                nc.sync.dma_start(W1b[k][:], W1bf[k * 128 : (k + 1) * 128, :])
            cst_sb = load(cst, [128, DH + 2 + 1 + DOUT + 1])
            wq_sb = load(wq, [128, NW * NG], BF16)
            W2b = [cp.tile([128, DH // 2], F32, name=f"w2b_{k}") for k in range(4)]
            for k in range(4):
                nc.sync.dma_start(W2b[k][:], W2[k * 128 : (k + 1) * 128, :])
            Wf1_sb = [cp.tile([128, DH // 4], F32, name=f"wf1_{k}") for k in range(2)]
            for k in range(2):
                nc.sync.dma_start(Wf1_sb[k][:], Wf1[k * 128 : (k + 1) * 128, :])
            idbf = cp.tile([128, 128], BF16)
            make_identity(nc, idbf[:])
            idf32 = cp.tile([128, 128], F32)
            make_identity(nc, idf32[:])

            b1r = cst_sb[:, 0:DH]
            b2c = cst_sb[:, DH : DH + 2]
            bf1c = cst_sb[:, DH + 2 : DH + 3]
            Wf2_sb = cst_sb[:, DH + 3 : DH + 3 + DOUT]
            bf2c = cst_sb[:DOUT, DH + 3 + DOUT : DH + 4 + DOUT]

            ga_local = dp.tile([NG, DH], F32)
            ga_red = dp.tile([NG, DH], F32, addr_space="Shared")
            gb_local = dp.tile([NG, DH], F32)
            gb_red = dp.tile([NG, DH], F32, addr_space="Shared")

            pgA = pp.tile([NG, DH], F32, tag="pgA", bufs=1, name="pgA")
            pgB = pp.tile([NG, DH], F32, tag="pgB", bufs=1, name="pgB")

            nm_tiles: dict = {}

            def emit_head(w):
                nch = int(K1[w])
                G1, oh = g1_t.pop(w), oh_t.pop(w)
                acc = pp.tile([128, DIN], F32, tag="acc", bufs=2, name=f"acc_{w}")
                for j in range(nch):
                    nc.tensor.matmul(
                        out=acc[:],
                        lhsT=oh[:, j, :],
                        rhs=G1[:, j, :],
                        start=(j == 0),
                        stop=(j == nch - 1),
                    )
                nm = wp.tile([128, DIN], BF16, tag="nm", bufs=3, name=f"nm_{w}")
                nc.scalar.activation(nm[:], acc[:], AF.Copy)
                nm_tiles[w] = nm

            def emit_tail(w):
                nm = nm_tiles.pop(w)
                sf = wp.tile([128, 2, 128], BF16, tag="sf", bufs=2, name=f"sf_{w}")
                for k in range(2):
                    pt = pp.tile([128, 128], BF16, tag="pt", bufs=2, name=f"pt_{w}_{k}")
                    nc.tensor.transpose(pt[:], nm[:, k * 128 : (k + 1) * 128], idbf[:])
                    nc.scalar.activation(sf[:, k, :], pt[:], AF.Copy)
                ph = pp.tile([128, DH], F32, tag="ph", bufs=2, name=f"ph_{w}")
                for k in range(2):
                    nc.tensor.matmul(
                        out=ph[:],
                        lhsT=sf[:, k, :],
                        rhs=W1b[k][:],
                        start=(k == 0),
                        stop=(k == 1),
                    )
                hb = wp.tile([128, DH], BF16, tag="hb", bufs=2, name=f"hb_{w}")
                nc.vector.tensor_tensor(out=hb[:], in0=ph[:], in1=b1r, op=OP.add)
                h1 = wp.tile([128, DH], BF16, tag="h1", bufs=2, name=f"h1_{w}")
                nc.scalar.activation(h1[:], hb[:], AF.Relu)
                pg = pgA if w < W_SPLIT else pgB
                w0, w1 = (0, W_SPLIT) if w < W_SPLIT else (W_SPLIT, NW)
                nc.tensor.matmul(
                    out=pg[:],
                    lhsT=wq_sb[:, w * NG : (w + 1) * NG],
                    rhs=h1[:],
                    start=(w == w0),
                    stop=(w == w1 - 1),
                )

            def emit_ar(pg, local, red, name):
                gsb = wp.tile([NG, DH], F32, name=f"gsb_{name}")
                nc.vector.tensor_copy(gsb[:], pg[:])
                nc.sync.dma_start(local[:], gsb[:])
                nc.gpsimd.collective_compute(
                    "AllReduce",
                    OP.add,
                    replica_groups=[list(range(NCORES))],
                    ins=[local.opt()],
                    outs=[red.opt()],
                )

            for w in range(NW + 1):
                if w < NW:
                    if w + PF < NW:
                        issue_dma(w + PF)
                    emit_head(w)
                if w >= 1:
                    emit_tail(w - 1)
                    if w - 1 == W_SPLIT - 1:
                        emit_ar(pgA, ga_local, ga_red, "a")
            emit_ar(pgB, gb_local, gb_red, "b")

            # ---------------- epilogue: combine + W2 + MLP ----------------
            ga_sb = wp.tile([NG, DH], F32)
            nc.sync.dma_start(ga_sb[:], ga_red[:])
            gb_sb = wp.tile([NG, DH], F32)
            nc.sync.dma_start(gb_sb[:], gb_red[:])
            gsum = wp.tile([NG, DH], F32)
            nc.vector.tensor_tensor(out=gsum[:], in0=ga_sb[:], in1=gb_sb[:], op=OP.add)

            ghT = [wp.tile([128, NG], F32, name=f"ghT_{k}") for k in range(4)]
            for k in range(4):
                pt = pp.tile([128, NG], F32, tag="pt", bufs=2, name=f"gt_{k}")
                nc.tensor.transpose(
                    pt[:], gsum[:, k * 128 : (k + 1) * 128], idf32[:NG, :NG]
                )
                nc.vector.tensor_copy(ghT[k][:], pt[:])
            g_fm = [wp.tile([128, NG], F32, name=f"gfm_{h}") for h in range(2)]
            for h in range(2):
                p2 = pp.tile([128, NG], F32, tag="acc", bufs=2, name=f"p2_{h}")
                for k in range(4):
                    nc.tensor.matmul(
                        out=p2[:],
                        lhsT=W2b[k][:, h * 128 : (h + 1) * 128],
                        rhs=ghT[k][:],
                        start=(k == 0),
                        stop=(k == 3),
                    )
                nc.scalar.activation(
                    g_fm[h][:], p2[:], AF.Relu, bias=b2c[:, h : h + 1]
                )
            pz = pp.tile([128, NG], F32, tag="ph", bufs=2, name="pz")
            for k in range(2):
                nc.tensor.matmul(
                    out=pz[:], lhsT=Wf1_sb[k][:], rhs=g_fm[k][:],
                    start=(k == 0), stop=(k == 1),
                )
            zsb = wp.tile([128, NG], F32)
            nc.scalar.activation(zsb[:], pz[:], AF.Relu, bias=bf1c)
            po = pp.tile([DOUT, NG], F32, tag="pt", bufs=2, name="po")
            nc.tensor.matmul(out=po[:], lhsT=Wf2_sb, rhs=zsb[:], start=True, stop=True)
            osb = wp.tile([DOUT, NG], F32)
            nc.scalar.activation(osb[:], po[:], AF.Relu, bias=bf2c)
            pout = pp.tile([NG, DOUT], F32, tag="pt", bufs=2, name="pout")
            nc.tensor.transpose(pout[:], osb[:], idf32[:DOUT, :DOUT])
            out_sb = wp.tile([NG, DOUT], F32)
            nc.vector.tensor_copy(out_sb[:], pout[:])
            nc.sync.dma_start(out[:], out_sb[:])

    nc.compile()
    return nc


def _get_program(meta):
    if meta not in _COMPILED:
        _COMPILED[meta] = _build_program(meta)
    return _COMPILED[meta]


def _make_in_maps(W1, b1, W2, b2, Wf1, bf1, Wf2, bf2, per_core):
    cstw = DH + 2 + 1 + DOUT + 1
    cst = np.zeros((128, cstw), dtype=np.float32)
    cst[:, 0:DH] = np.asarray(b1, np.float32)[None, :]
    cst[:, DH : DH + 2] = np.asarray(b2, np.float32).reshape(2, 128).T
    cst[:, DH + 2] = np.asarray(bf1, np.float32)
    cst[:, DH + 3 : DH + 3 + DOUT] = np.asarray(Wf2, np.float32)
    cst[:DOUT, DH + 3 + DOUT] = np.asarray(bf2, np.float32)
    shared = dict(
        cst=cst,
        W1bf=np.asarray(W1, np.float32).astype(ml_dtypes.bfloat16),
        W2=np.asarray(W2, np.float32),
        Wf1=np.asarray(Wf1, np.float32),
    )
    return [dict(shared, **per_core[c]) for c in range(NCORES)]


def kernel(
    x, W1, b1, W2, b2, Wf1, bf1, Wf2, bf2, edge_index, batch, num_graphs, _trace=False
):
    assert int(num_graphs) == NG
    meta, per_core = _preprocess(np.asarray(x), np.asarray(edge_index), np.asarray(batch))
    nc = _get_program(meta)
    in_maps = _make_in_maps(W1, b1, W2, b2, Wf1, bf1, Wf2, bf2, per_core)
    res = bass_utils.run_bass_kernel_spmd(
        nc, in_maps, core_ids=list(range(NCORES)), trace=_trace
    )
    out = np.asarray(res.results[0]["out"], np.float32)
    if _trace:
        kernel._last_results = res
    return out
